# revision 4
# baseline (speedup 1.0000x reference)
"""Trainium2 Bass kernel: streaming Conv-TasNet separation step (single core).

Pipeline (channel-major, fp16 TCN activations):
  encoder conv + front-end cLN + bottleneck (fp32, tiny)
  14 dilated TCN blocks:
    W1 1x1 (PE, fp16) -> PReLU (ACT Prelu, runtime alpha) -> cLN stats via
    all-ones matmul (mean broadcast in PSUM) + Square/Sqrt/reciprocal ->
    u1 = z*r -> depthwise dilated conv as 3 fused scalar_tensor_tensor taps
    (cLN mean correction enters as rank-1 K=1 matmuls in the first tap's
    tensor slot; cLN affine folded into dw weights/bias host-side) ->
    PReLU2 -> cLN2 (same trick; affine folded into W2) -> W2 1x1 (PE) with
    rank-1 mean term -> bias + residual in one fused op.
  mask 1x1 + ReLU, decoder basis matmul (fp32, tiny)
Ring-buffer shifts are pure data movement and happen host-side.
"""
import os
import numpy as np

# ---- problem constants (hardcoded per contract) ----
CHUNK, L, JUMP, NCH, BCH, H, KW = 400, 50, 8, 128, 128, 512, 3
R, X, BUF, EPS = 2, 7, 516, 1e-8
NB = R * X
DILS = [2 ** (i % X) for i in range(NB)]
T_INS = []
_t = BUF
for _d in DILS:
    T_INS.append(_t)
    _t -= 2 * _d
assert _t == JUMP

SEG_CAP = 512  # PSUM bank capacity in fp32 elements per partition

_PROG = None  # (nc, meta) cache
LAST_EXEC_NS = None


def _segs(total, cap=SEG_CAP):
    n = -(-total // cap)
    base = -(-total // n)
    out = []
    o = 0
    while o < total:
        ln = min(base, total - o)
        out.append((o, ln))
        o += ln
    return out


# ---------------- host-side weight preparation ----------------
def _host_prep(inputs):
    f32 = lambda a: np.ascontiguousarray(np.asarray(a, np.float32))
    f16 = lambda a: np.ascontiguousarray(np.asarray(a, np.float32).astype(np.float16))
    inp = {k: np.asarray(v, np.float32) for k, v in inputs.items()}

    mix = inp['mixture'][0]                      # [2, 400]
    mix_unf = mix.reshape(2, JUMP, L).transpose(0, 2, 1).reshape(100, JUMP)
    wencT = inp['W_enc'].transpose(1, 2, 0).reshape(100, NCH)
    W_bng = inp['W_bn'] * inp['ln_g'][None, :]
    cbnb = (inp['W_bn'] @ inp['ln_b'])[:, None]

    w1T = np.empty((BCH, NB * H), np.float16)
    w2gT = np.empty((128, NB * 4 * 128), np.float16)
    negc = np.empty((1, NB * 3 * 4 * 128), np.float16)
    w2gsneg = np.empty((1, NB * 128), np.float16)
    # per-partition fp32 scalars, packed: per block: [a1, a2, cbn2, then per
    # chunk c: c0, c1, c2, cb1]  -> 3 + 16 = 19 cols per block
    NSC = NB * 19
    scal = np.zeros((128, NSC), np.float32)

    for i in range(NB):
        W1 = inp['blk_w1'][i]                    # [512, 128]
        g1, b1 = inp['blk_g1'][i], inp['blk_b1'][i]
        w = inp['blk_dw'][i]                     # [512, 3]
        g2, b2 = inp['blk_g2'][i], inp['blk_b2'][i]
        W2 = inp['blk_w2'][i]                    # [128, 512]
        wg = w * g1[:, None]                     # [512, 3]
        cb1 = b1 * w.sum(1)                      # [512]
        W2g = (W2 * g2[None, :]).astype(np.float16)
        cbn2 = W2 @ b2                           # [128]

        w1T[:, i * H:(i + 1) * H] = W1.T.astype(np.float16)
        for c in range(4):
            w2gT[:, (i * 4 + c) * 128:(i * 4 + c + 1) * 128] = \
                W2g[:, c * 128:(c + 1) * 128].T
            for k in range(3):
                negc[0, ((i * 3 + k) * 4 + c) * 128:((i * 3 + k) * 4 + c + 1) * 128] = \
                    (-wg[c * 128:(c + 1) * 128, k]).astype(np.float16)
        w2gsneg[0, i * 128:(i + 1) * 128] = (-W2g.astype(np.float32).sum(1)).astype(np.float16)

        col = i * 19
        scal[:, col + 0] = inp['blk_p1'][i]
        scal[:, col + 1] = inp['blk_p2'][i]
        scal[:, col + 2] = cbn2
        for c in range(4):
            cc = col + 3 + c * 4
            scal[:, cc + 0] = wg[c * 128:(c + 1) * 128, 0]
            scal[:, cc + 1] = wg[c * 128:(c + 1) * 128, 1]
            scal[:, cc + 2] = wg[c * 128:(c + 1) * 128, 2]
            scal[:, cc + 3] = cb1[c * 128:(c + 1) * 128]

    return {
        'mix_unf': f32(mix_unf),
        'wencT': f32(wencT),
        'wbngT': f32(W_bng.T),
        'cbnb': f32(cbnb),
        'ones128': np.full((128, 128), 1.0 / 128, np.float32),
        'negone': np.full((1, 128), -1.0, np.float32),
        'epsv': np.full((128, 1), EPS, np.float32),
        'lnbuf_tail': f32(inp['layernorm_buffer'][0][:, JUMP:]),
        'scal': scal,
        'ones512': np.full((128, 128), 1.0 / 512, np.float16),
        'w1T': w1T,
        'w2gT': w2gT,
        'negc': negc,
        'w2gsneg': w2gsneg,
        'wmaskT': f16(inp['W_mask'].T),
        'wdecT': f32(inp['W_dec'].T),
    }


# ---------------- device program ----------------
def _build():
    import concourse.bass as bass
    import concourse.mybir as mybir
    import concourse.tile as tile
    from concourse import bacc

    F32, F16 = mybir.dt.float32, mybir.dt.float16
    AF = mybir.ActivationFunctionType
    OP = mybir.AluOpType

    nc = bacc.Bacc("TRN2", target_bir_lowering=False, debug=False,
                   enable_asserts=False)

    dt_shapes = dict(
        mix_unf=('f32', (100, JUMP)), wencT=('f32', (100, NCH)),
        wbngT=('f32', (128, 128)), cbnb=('f32', (128, 1)),
        ones128=('f32', (128, 128)), negone=('f32', (1, 128)),
        epsv=('f32', (128, 1)), lnbuf_tail=('f32', (128, BUF - JUMP)),
        scal=('f32', (128, NB * 19)), ones512=('f16', (128, 128)),
        w1T=('f16', (BCH, NB * H)), w2gT=('f16', (128, NB * 4 * 128)),
        negc=('f16', (1, NB * 3 * 4 * 128)), w2gsneg=('f16', (1, NB * 128)),
        wmaskT=('f16', (128, 128)), wdecT=('f32', (128, 50)),
    )
    dram = {}
    for name, (dt, shape) in dt_shapes.items():
        dram[name] = nc.dram_tensor(name, list(shape),
                                    F32 if dt == 'f32' else F16,
                                    kind="ExternalInput").ap()
    enc_out = nc.dram_tensor('enc_out', [128, JUMP], F32, kind="ExternalOutput").ap()
    ln_new_out = nc.dram_tensor('ln_new_out', [128, JUMP], F32, kind="ExternalOutput").ap()
    est_out = nc.dram_tensor('est_out', [JUMP, 50], F32, kind="ExternalOutput").ap()

    with tile.TileContext(nc) as tc:
        with (
            tc.tile_pool(name="wt", bufs=1) as wt,
            tc.tile_pool(name="xb", bufs=2) as xb,
            tc.tile_pool(name="acts", bufs=2) as acts,
            tc.tile_pool(name="tmp", bufs=3) as tmp,
            tc.tile_pool(name="st", bufs=2) as st,
            tc.tile_pool(name="ps_y", bufs=2, space="PSUM") as ps_y,
            tc.tile_pool(name="ps_mu", bufs=1, space="PSUM") as ps_mu,
            tc.tile_pool(name="ps_var", bufs=1, space="PSUM") as ps_var,
            tc.tile_pool(name="ps_corr", bufs=2, space="PSUM") as ps_corr,
            tc.tile_pool(name="ps_w2", bufs=1, space="PSUM") as ps_w2,
        ):
            # ---- load weights/constants to SBUF ----
            sb = {}
            for name, (dt, shape) in dt_shapes.items():
                t = wt.tile(list(shape), F32 if dt == 'f32' else F16, tag=name)
                nc.sync.dma_start(t[:], dram[name][:, :])
                sb[name] = t

            def sc_ap(i, j):   # per-partition fp32 scalar column
                return sb['scal'][:, i * 19 + j: i * 19 + j + 1]

            lp = nc.allow_low_precision

            # ---- front-end: encoder + cLN + bottleneck (fp32, T=8) ----
            enc_ps = ps_y.tile([128, JUMP], F32, tag='y')
            nc.tensor.matmul(enc_ps[:], sb['wencT'][:], sb['mix_unf'][:],
                             start=True, stop=True)
            enc_sb = st.tile([128, JUMP], F32, tag='enc')
            nc.scalar.activation(enc_sb[:], enc_ps[:], AF.Relu)
            nc.sync.dma_start(enc_out[:, :], enc_sb[:])

            encsq = tmp.tile([128, JUMP], F32, tag='fsq')
            nc.vector.tensor_tensor(encsq[:], enc_sb[:], enc_sb[:], op=OP.mult)
            mu_ps = ps_mu.tile([128, JUMP], F32, tag='mu')
            nc.tensor.matmul(mu_ps[:], sb['ones128'][:], enc_sb[:], start=True, stop=True)
            e2_ps = ps_var.tile([128, JUMP], F32, tag='var')
            nc.tensor.matmul(e2_ps[:], sb['ones128'][:], encsq[:], start=True, stop=False)
            musq = st.tile([1, JUMP], F32, tag='musq')
            nc.scalar.activation(musq[:], mu_ps[0:1, :], AF.Square)
            nc.tensor.matmul(e2_ps[:], sb['negone'][:], musq[:], start=False, stop=True)
            rt = st.tile([128, JUMP], F32, tag='frt')
            nc.scalar.activation(rt[:], e2_ps[:], AF.Sqrt, bias=sb['epsv'][:])
            r_fe = st.tile([128, JUMP], F32, tag='fr')
            nc.vector.reciprocal(r_fe[:], rt[:])
            t1 = tmp.tile([128, JUMP], F32, tag='ft1')
            nc.vector.tensor_tensor(t1[:], enc_sb[:], mu_ps[:], op=OP.subtract)
            u_fe = tmp.tile([128, JUMP], F32, tag='fu')
            nc.vector.tensor_tensor(u_fe[:], t1[:], r_fe[:], op=OP.mult)
            ln_ps = ps_w2.tile([128, JUMP], F32, tag='w2')
            nc.tensor.matmul(ln_ps[:], sb['wbngT'][:], u_fe[:], start=True, stop=True)
            ln_new = st.tile([128, JUMP], F32, tag='lnnew')
            nc.scalar.activation(ln_new[:], ln_ps[:], AF.Identity,
                                 bias=sb['cbnb'][:], scale=1.0)
            nc.sync.dma_start(ln_new_out[:, :], ln_new[:])

            # ---- TCN input assembly ----
            x_cur = xb.tile([128, BUF], F16, tag='x')
            with lp(reason="fp16 TCN activations"):
                nc.vector.tensor_copy(x_cur[:, 0:BUF - JUMP], sb['lnbuf_tail'][:])
                nc.vector.tensor_copy(x_cur[:, BUF - JUMP:BUF], ln_new[:])

            # ---- TCN blocks ----
            for i in range(NB):
                d = DILS[i]
                T = T_INS[i]
                Tp = T - 2 * d
                z = acts.tile([128, 4, T], F16, tag='z')
                u1 = acts.tile([128, 4, T], F16, tag='u1')
                q = acts.tile([128, 4, Tp], F16, tag='q')
                u2 = acts.tile([128, 4, Tp], F16, tag='u2')
                r1 = st.tile([128, T], F16, tag='r1')
                m1 = st.tile([128, T], F16, tag='m1')
                r2 = st.tile([128, Tp], F16, tag='r2')
                m2 = st.tile([128, Tp], F16, tag='m2')

                # phase 1: W1 + PReLU1 + stats -> r1, m1, u1
                for (o, ln) in _segs(T):
                    mu1_ps = ps_mu.tile([128, ln], F32, tag='mu')
                    e21_ps = ps_var.tile([128, ln], F32, tag='var')
                    for c in range(4):
                        y_ps = ps_y.tile([128, ln], F32, tag='y')
                        nc.tensor.matmul(
                            y_ps[:], sb['w1T'][:, i * H + c * 128: i * H + (c + 1) * 128],
                            x_cur[:, o:o + ln], start=True, stop=True)
                        with lp(reason="fp16 acts"):
                            nc.scalar.activation(z[:, c, o:o + ln], y_ps[:], AF.Prelu,
                                                 scale=1.0, alpha=sc_ap(i, 0))
                        zsq = tmp.tile([128, ln], F16, tag='zsq')
                        eng = nc.gpsimd if c == 3 else nc.vector
                        with lp(reason="fp16 acts"):
                            eng.tensor_tensor(zsq[:], z[:, c, o:o + ln],
                                              z[:, c, o:o + ln], op=OP.mult)
                        nc.tensor.matmul(mu1_ps[:], sb['ones512'][:], z[:, c, o:o + ln],
                                         start=(c == 0), stop=(c == 3))
                        nc.tensor.matmul(e21_ps[:], sb['ones512'][:], zsq[:],
                                         start=(c == 0), stop=False)
                    musq1 = st.tile([1, ln], F32, tag='musq')
                    nc.scalar.activation(musq1[:], mu1_ps[0:1, :], AF.Square)
                    nc.tensor.matmul(e21_ps[:], sb['negone'][:], musq1[:],
                                     start=False, stop=True)
                    rt1 = st.tile([128, ln], F32, tag='rt')
                    nc.scalar.activation(rt1[:], e21_ps[:], AF.Sqrt, bias=sb['epsv'][:])
                    with lp(reason="fp16 layernorm scale"):
                        nc.vector.reciprocal(r1[:, o:o + ln], rt1[:])
                        nc.vector.tensor_tensor(m1[:, o:o + ln], mu1_ps[:],
                                                r1[:, o:o + ln], op=OP.mult)
                        for c in range(4):
                            eng = nc.gpsimd if c == 3 else nc.vector
                            eng.tensor_tensor(u1[:, c, o:o + ln], z[:, c, o:o + ln],
                                              r1[:, o:o + ln], op=OP.mult)

                # phase 2: dwconv (3 fused taps + rank-1 mean corr) + PReLU2 + stats2
                for (o, ln) in _segs(Tp):
                    mu2_ps = ps_mu.tile([128, ln], F32, tag='mu')
                    e22_ps = ps_var.tile([128, ln], F32, tag='var')
                    for c in range(4):
                        corr_ps = ps_corr.tile([128, ln], F32, tag='corr')
                        for k in range(3):
                            nb = ((i * 3 + k) * 4 + c) * 128
                            nc.tensor.matmul(corr_ps[:], sb['negc'][0:1, nb:nb + 128],
                                             m1[0:1, o + k * d: o + k * d + ln],
                                             start=(k == 0), stop=(k == 2))
                        cc = 3 + c * 4
                        A = tmp.tile([128, ln], F16, tag='A')
                        A2 = tmp.tile([128, ln], F16, tag='A2')
                        Bt = tmp.tile([128, ln], F16, tag='B')
                        with lp(reason="fp16 acts"):
                            nc.vector.scalar_tensor_tensor(
                                A[:], u1[:, c, o:o + ln], sc_ap(i, cc + 0), corr_ps[:],
                                op0=OP.mult, op1=OP.add)
                            nc.vector.scalar_tensor_tensor(
                                A2[:], u1[:, c, o + d:o + d + ln], sc_ap(i, cc + 1), A[:],
                                op0=OP.mult, op1=OP.add)
                            nc.vector.scalar_tensor_tensor(
                                Bt[:], u1[:, c, o + 2 * d:o + 2 * d + ln], sc_ap(i, cc + 2),
                                A2[:], op0=OP.mult, op1=OP.add)
                            nc.scalar.activation(q[:, c, o:o + ln], Bt[:], AF.Prelu,
                                                 bias=sc_ap(i, cc + 3), scale=1.0,
                                                 alpha=sc_ap(i, 1))
                        qsq = tmp.tile([128, ln], F16, tag='zsq')
                        eng = nc.gpsimd if c == 3 else nc.vector
                        with lp(reason="fp16 acts"):
                            eng.tensor_tensor(qsq[:], q[:, c, o:o + ln],
                                              q[:, c, o:o + ln], op=OP.mult)
                        nc.tensor.matmul(mu2_ps[:], sb['ones512'][:], q[:, c, o:o + ln],
                                         start=(c == 0), stop=(c == 3))
                        nc.tensor.matmul(e22_ps[:], sb['ones512'][:], qsq[:],
                                         start=(c == 0), stop=False)
                    musq2 = st.tile([1, ln], F32, tag='musq')
                    nc.scalar.activation(musq2[:], mu2_ps[0:1, :], AF.Square)
                    nc.tensor.matmul(e22_ps[:], sb['negone'][:], musq2[:],
                                     start=False, stop=True)
                    rt2 = st.tile([128, ln], F32, tag='rt')
                    nc.scalar.activation(rt2[:], e22_ps[:], AF.Sqrt, bias=sb['epsv'][:])
                    with lp(reason="fp16 layernorm scale"):
                        nc.vector.reciprocal(r2[:, o:o + ln], rt2[:])
                        nc.vector.tensor_tensor(m2[:, o:o + ln], mu2_ps[:],
                                                r2[:, o:o + ln], op=OP.mult)
                        for c in range(4):
                            eng = nc.gpsimd if c == 3 else nc.vector
                            eng.tensor_tensor(u2[:, c, o:o + ln], q[:, c, o:o + ln],
                                              r2[:, o:o + ln], op=OP.mult)

                # phase 3: W2 + rank-1 mean + bias + residual
                x_next = xb.tile([128, Tp], F16, tag='x')
                for (o, ln) in _segs(Tp):
                    w2_ps = ps_w2.tile([128, ln], F32, tag='w2')
                    for c in range(4):
                        nb = (i * 4 + c) * 128
                        nc.tensor.matmul(w2_ps[:], sb['w2gT'][:, nb:nb + 128],
                                         u2[:, c, o:o + ln], start=(c == 0), stop=False)
                    nc.tensor.matmul(w2_ps[:], sb['w2gsneg'][0:1, i * 128:(i + 1) * 128],
                                     m2[0:1, o:o + ln], start=False, stop=True)
                    with lp(reason="fp16 acts"):
                        nc.vector.scalar_tensor_tensor(
                            x_next[:, o:o + ln], w2_ps[:], sc_ap(i, 2),
                            x_cur[:, 2 * d + o: 2 * d + o + ln],
                            op0=OP.add, op1=OP.add)
                x_cur = x_next

            # ---- mask + decoder ----
            mask_ps = ps_y.tile([128, JUMP], F32, tag='y')
            nc.tensor.matmul(mask_ps[:], sb['wmaskT'][:], x_cur[:, 0:JUMP],
                             start=True, stop=True)
            mask_sb = tmp.tile([128, JUMP], F32, tag='mask')
            nc.scalar.activation(mask_sb[:], mask_ps[:], AF.Relu)
            srcw = tmp.tile([128, JUMP], F32, tag='srcw')
            nc.vector.tensor_tensor(srcw[:], enc_sb[:], mask_sb[:], op=OP.mult)
            est_ps = ps_w2.tile([JUMP, 50], F32, tag='w2')
            nc.tensor.matmul(est_ps[:], srcw[:], sb['wdecT'][:], start=True, stop=True)
            est_sb = tmp.tile([JUMP, 50], F32, tag='est')
            nc.vector.tensor_copy(est_sb[:], est_ps[:])
            nc.sync.dma_start(est_out[:, :], est_sb[:])

    nc.compile()
    return nc


def _get_prog():
    global _PROG
    if _PROG is None:
        _PROG = _build()
    return _PROG


def _host_post(dev, inputs):
    est = np.asarray(dev['est_out'], np.float32).reshape(1, 1, JUMP * L)
    enc = np.asarray(dev['enc_out'], np.float32)
    ln_new = np.asarray(dev['ln_new_out'], np.float32)
    encoder_buffer = np.concatenate(
        [np.asarray(inputs['encoder_buffer'], np.float32)[:, :, JUMP:], enc[None]], 2)
    layernorm_buffer = np.concatenate(
        [np.asarray(inputs['layernorm_buffer'], np.float32)[:, :, JUMP:], ln_new[None]], 2)
    return est, encoder_buffer, layernorm_buffer


def _install_ntff_hook():
    """The image's antenv lacks axon_hooks; synthesize it and register the
    ctypes NTFF hook so run_bass_kernel_spmd(trace=True) can profile."""
    import sys
    import types
    if 'antenv.axon_hooks' in sys.modules:
        return True
    try:
        import antenv
        from trn_agent_boot.trn_boot import _ntff_profile_via_ctypes
        hook = _ntff_profile_via_ctypes('/opt/axon/libaxon_pjrt.so')
        if hook is None:
            return False
        mod = types.ModuleType('antenv.axon_hooks')
        mod._hook = hook
        mod.set_axon_ntff_profile_hook = lambda h: setattr(mod, '_hook', h)
        mod.get_axon_ntff_profile_hook = lambda: mod._hook
        sys.modules['antenv.axon_hooks'] = mod
        antenv.axon_hooks = mod
        return True
    except Exception:
        return False


def kernel(**inputs):
    global LAST_EXEC_NS
    from concourse import bass_utils
    nc = _get_prog()
    in_map = _host_prep(inputs)
    trace = bool(int(os.environ.get('KERNEL_TRACE', '0')))
    if trace:
        trace = _install_ntff_hook()
    res = bass_utils.run_bass_kernel_spmd(nc, [in_map], core_ids=[0], trace=trace)
    LAST_EXEC_NS = res.exec_time_ns
    return _host_post(res.results[0], inputs)
